# revision 57
# baseline (speedup 1.0000x reference)
# Masked multi-head attention for Trainium2, SPMD over 8 NeuronCores.
#
# Problem: q,k,v [2,16,2048,64] f32, mask [1,1,2048,2048] int32 (0/1),
#   out[b,h] = softmax(q@k^T/8 masked) @ v.
#
# Sharding: B*H = 32 heads, 4 per core (embarrassingly parallel).
#
# Per-head on-chip algorithm (no max-subtraction needed: scores ~ N(0,1),
# exp never overflows fp32; masked softmax == exp(S)*mask / sum(exp(S)*mask)):
#   Work in the transposed orientation S^T[k,q] so the softmax reduction
#   (over k) lands on the PE contraction dim instead of needing a
#   partition-axis reduction:
#     S^T[kc] (psum)  = kT[:,kc].T @ qT            (kc = 16 chunks of 128 k)
#     P^T[kc] (sbuf)  = exp(S^T[kc])          [ScalarE, psum->sbuf, fp16]
#     P^T[kc]        *= maskT[kc]             [VectorE, fp16 2x mode]
#     acc[65,2048]   += vp[kc].T @ P^T[kc]    [vp = [V | ones], fp32 psum]
#   acc rows 0..63 = (P@V)^T, row 64 = l = sum_k P.  Host divides and
#   transposes back.  The 1/sqrt(64) scale is folded into qT on the host.
#
# Pipeline structure (the ScalarE exp stream, 0.83ns/elem, is the roofline:
# ~134us busy per core; everything else must hide under it):
#   - PV_DELAY=2 + PV_ILV + PV_FIRST: each pending PV pair is emitted just
#     before the spans QK pair two chunks later, so the in-order PE never waits
#     on the exp->mask chain and the PE stays continuously fed.  (The early
#     re-emissions of pending[0] at chunks 1-3 look redundant but are load-
#     bearing on HW: removing them drops the PE clock / p-state.)
#   - a dma_start occupies its issuing engine ~1us, so ALL bulk DMA (mask
#     8 MiB/iter, vp, out stores) issues from the otherwise-idle Pool engine
#     via SWDGE; SP carries only the one merged qT|kT load per head, issued
#     ahead of the mask burst (IO_FIRST + QK_MERGE).
#   - the bench For_i body is unrolled 5x to amortize the loop's all-engine
#     barrier + semaphore reset (a full pipeline drain per iteration).
import os
from contextlib import ExitStack

import numpy as np
import ml_dtypes

B, H, S, D = 2, 16, 2048, 64
N_CORES = 8
HPC = (B * H) // N_CORES  # heads per core = 4
P = 128
NCHUNK = S // P  # 16


def _env(name, default):
    v = os.environ.get(name)
    if v is None:
        return default
    if isinstance(default, bool):
        return v not in ("0", "", "false", "False")
    if isinstance(default, int):
        return int(v)
    return v


# Precision / tiling knobs.  NOTE: TRN2 matmul output must be fp32, so score
# psum is always f32: span=1024 keeps psum at 2(st)x2bufs + 4(acc) = 8 banks.
# fp16 measured ~6x more accurate than bf16 at identical HW speed
# (rel absmax 8.2e-4 vs 4.7e-3 across all heads).
PREC = _env("K_PREC", "fp16")  # 16-bit dtype for qT/kT/vp/maskT/P
QK_DT = _env("K_QK_DT", "16")  # dtype of qT/kT fed to the PE ("16" | "f32r" | "f32")
QK_PACK = _env("K_QK_PACK", False)  # row-tile the QK matmuls: 2 chunks concurrently
QK_MERGE = _env("K_QK_MERGE", True)  # qT+kT in one [64,2S] tile / one DMA per head
AUX_GPS = _env("K_AUX_GPS", True)  # vp loads + out stores via Pool SWDGE too
MASK_SEP = _env("K_MASK_SEP", False)  # mask-multiply into a separate tile
PV_DELAY = _env("K_PV_DELAY", 2)  # emit chunk c's PV after QK of c+PV_DELAY
MM_N = _env("K_MM_N", 512)  # matmul output free width (512 = one psum bank)
PT_BUFS = _env("K_PT_BUFS", 10)  # pt pool slots (ACT->DVE->PE pipeline depth)
IO_BUFS = _env("K_IO_BUFS", 2)  # per-head qT/kT/vp prefetch depth
OUT_BUFS = _env("K_OUT_BUFS", 2)  # out_sb ring depth (copy -> store chain)
MASK_SPARE = _env("K_MASK_SPARE", 2)  # mask pool bufs beyond NCHUNK
MASK_GPS = _env("K_MASK_GPS", False)  # route every second mask-multiply to GpSimd
PV_ILV = _env("K_PV_ILV", True)  # interleave delayed PV per-span with QK
PV_FIRST = _env("K_PV_FIRST", True)  # emit the pending PV pair before the span's QKs
PV_ONCE = _env("K_PV_ONCE", False)  # emit each chunk's PV exactly once + spread tail
PV_TAIL = _env("K_PV_TAIL", False)  # spread the head-end PV tail burst over chunks 13-15
WARM = _env("K_WARM", 0)  # post-barrier PE warm-up matmuls (bench loop only)
MASK_WIDE = _env("K_MASK_WIDE", False)  # one FD-2048 mask TT per chunk
OUT_ENG = _env("K_OUT_ENG", "vector")  # engine for the psum->sbuf out copy
OUT_SPLIT = _env("K_OUT_SPLIT", False)  # defer+halve out copies into next head
IO_FIRST = _env("K_IO_FIRST", True)  # issue head-0 io DMAs ahead of the mask burst
MASK_DMA_ENG = _env("K_MASK_DMA_ENG", "gps")  # all mask loads via Pool SWDGE
# (a hwdge dma_start occupies the issuing engine ~1us: on ACT that comes off the
# exp roofline, on SP it delays the io loads behind the 8 MiB mask burst.  The
# Pool engine is otherwise idle, so it issues all 16 mask DMAs for free.)
STAGGER = _env("K_STAGGER", False)  # staggered sem-reset For_i (bench loop only)
LOOP_HINTS = _env("K_LOOP_HINTS", False)  # branch-prefetch hints on the For_i back-edge
UNROLL = _env("K_UNROLL", 5)  # bodies per For_i iteration (bench loop only)
# ablation knobs (bench-only attribution experiments; break correctness)
MASK_OFF = _env("K_MASK_OFF", False)
PV_OFF = _env("K_PV_OFF", False)
QK_OFF = _env("K_QK_OFF", False)
MASK_DMA_ONCE = _env("K_MASK_DMA_ONCE", False)  # bench-only: hoist mask DMA out of For_i

_CACHE = {}
LAST_RESULT = None  # BassKernelResults of the most recent run (for test.py)


def _build_nc(loop_reps=None):
    """Build the Bass program.  loop_reps=None -> the real kernel;
    loop_reps=K wraps the whole body in a hardware For_i loop (bench-only:
    lets wall-clock diffs between two K values measure per-iteration HW
    time through the slow axon tunnel)."""
    import concourse.bass as bass
    import concourse.tile as tile
    from concourse import bacc, mybir

    DT16 = mybir.dt.float16 if PREC == "fp16" else mybir.dt.bfloat16
    F32 = mybir.dt.float32
    qk_mm_dt = {"16": DT16, "f32r": mybir.dt.float32r, "f32": F32}[QK_DT]
    score_dt = F32
    # one matmul output must fit in one psum bank (512 fp32/partition)
    qk_n = MM_N
    # one score tile: free-dim span of a single exp instruction
    span = 1024
    spans = S // span

    nc = bacc.Bacc("TRN2", target_bir_lowering=False, debug=False)

    qk_rows = 128 if QK_PACK else 64
    if QK_MERGE:
        # qT and kT side by side along the free dim (same partitions 0-63,
        # as the PE needs both operands at the same base partition)
        qk = nc.dram_tensor(
            "qk", [HPC, qk_rows, 2 * S], qk_mm_dt, kind="ExternalInput"
        ).ap()
        qT = kT = None
    else:
        qT = nc.dram_tensor("qT", [HPC, qk_rows, S], qk_mm_dt, kind="ExternalInput").ap()
        kT = nc.dram_tensor("kT", [HPC, qk_rows, S], qk_mm_dt, kind="ExternalInput").ap()
    vp = nc.dram_tensor("vp", [HPC, S, D + 1], DT16, kind="ExternalInput").ap()
    maskT = nc.dram_tensor("maskT", [S, S], DT16, kind="ExternalInput").ap()
    o = nc.dram_tensor("o", [HPC, D + 1, S], F32, kind="ExternalOutput").ap()

    with tile.TileContext(nc) as tc, ExitStack() as ctx:
        warm_ab = None
        if WARM and loop_reps is not None:
            # constant tiles with no DMA dependency: warm matmuls issued right
            # after the For_i barrier ramp the PE clock while real io loads fly
            wp = ctx.enter_context(tc.tile_pool(name="warm", bufs=1))
            warm_a = wp.tile([64, P], qk_mm_dt, tag="wa", name="warm_a")
            nc.gpsimd.memset(warm_a[:], 0.0)
            warm_b = wp.tile([64, qk_n], qk_mm_dt, tag="wb", name="warm_b")
            nc.gpsimd.memset(warm_b[:], 0.0)
            warm_ab = (warm_a, warm_b)
        mask_pool = ctx.enter_context(tc.tile_pool(name="mask", bufs=NCHUNK + MASK_SPARE))
        io_pool = ctx.enter_context(tc.tile_pool(name="io", bufs=IO_BUFS))
        pt_pool = ctx.enter_context(tc.tile_pool(name="pt", bufs=PT_BUFS))
        out_pool = ctx.enter_context(tc.tile_pool(name="outsb", bufs=OUT_BUFS))
        qk_psum = ctx.enter_context(tc.tile_pool(name="qk_psum", bufs=2, space="PSUM"))
        acc_psum = ctx.enter_context(tc.tile_pool(name="acc_psum", bufs=1, space="PSUM"))

        def mask_dma_eng(c):
            if MASK_DMA_ENG == "split":
                return nc.scalar if c % 2 == 0 else nc.sync
            if MASK_DMA_ENG == "gsplit":
                # keep the ACT engine free of DMA issues: Pool SWDGE + SP
                return nc.gpsimd if c % 2 == 0 else nc.sync
            return {"act": nc.scalar, "sp": nc.sync, "gps": nc.gpsimd}[MASK_DMA_ENG]

        def load_mask():
            # mask^T resident in SBUF for all heads, one tile per k-chunk
            # (bufs = NCHUNK+2 so a following iteration's reload can start
            # while late chunks of the previous one are still being read).
            # Issued on a separate hwdge queue so the 8 MiB burst doesn't
            # sit ahead of the per-head qT/kT/vp loads on the SP queue.
            mt = maskT.rearrange("(c p) q -> p c q", p=P)
            tiles = []
            for c in range(NCHUNK):
                mtile = mask_pool.tile([P, S], DT16, tag="mchunk", name=f"mask_c{c}")
                mask_dma_eng(c).dma_start(mtile[:], mt[:, c, :])
                tiles.append(mtile)
            return tiles

        hoisted = [None]

        def load_io(h):
            if QK_MERGE:
                qk_sb = io_pool.tile([qk_rows, 2 * S], qk_mm_dt, tag="qk", name=f"qk_sb{h}")
                nc.sync.dma_start(qk_sb[:], qk[h])
                qT_sb = qk_sb[:, 0:S]
                kT_sb = qk_sb[:, S : 2 * S]
            else:
                qT_sb = io_pool.tile([qk_rows, S], qk_mm_dt, tag="qT", name=f"qT_sb{h}")
                nc.sync.dma_start(qT_sb[:], qT[h])
                kT_sb = io_pool.tile([qk_rows, S], qk_mm_dt, tag="kT", name=f"kT_sb{h}")
                nc.sync.dma_start(kT_sb[:], kT[h])
            vp_sb = io_pool.tile([P, NCHUNK, D + 1], DT16, tag="vp", name=f"vp_sb{h}")
            vp_eng = nc.gpsimd if AUX_GPS else nc.sync
            vp_eng.dma_start(vp_sb[:], vp[h].rearrange("(c p) d -> p c d", p=P))
            return qT_sb, kT_sb, vp_sb

        def body(_iv=None, warm=False):
            pre_io = load_io(0) if IO_FIRST else None
            if MASK_OFF:
                maskT_sb = None  # pure QK+exp(+PV) attribution: no mask load
            else:
                maskT_sb = hoisted[0] if hoisted[0] is not None else load_mask()
            _heads(maskT_sb, pre_io, warm)

        def _heads(maskT_sb, pre_io=None, warm=False):
          deferred = []  # [(acc, out_sb, h)] copies postponed into the next head

          def out_eng():
              return {"pool": nc.gpsimd, "vector": nc.vector, "scalar": nc.scalar}[
                  OUT_ENG
              ]

          def flush_deferred(stage):
              # stage 0/1: copy one half of the deferred head's acc; DMA after 1
              if not deferred:
                  return
              accp, out_sb, hh = deferred[0]
              half = S // 2
              sl = slice(stage * half, (stage + 1) * half)
              out_eng().tensor_copy(out_sb[:, sl], accp[:, sl])
              if stage == 1:
                  (nc.gpsimd if AUX_GPS else nc.sync).dma_start(o[hh], out_sb[:])
                  deferred.pop(0)

          for h in range(HPC):
            if h == 0 and pre_io is not None:
                qT_sb, kT_sb, vp_sb = pre_io
            else:
                qT_sb, kT_sb, vp_sb = load_io(h)

            acc = None
            if not PV_OFF:
                acc = acc_psum.tile([D + 1, S], F32, tag="acc", name=f"acc{h}")

            def emit_pv(c, pts):
                if PV_OFF:
                    return
                for sp in range(spans):
                    for qs in range(span // MM_N):
                        q0 = sp * span + qs * MM_N
                        nc.tensor.matmul(
                            acc[:, q0 : q0 + MM_N],
                            lhsT=vp_sb[:, c, :],
                            rhs=pts[sp][:, qs * MM_N : (qs + 1) * MM_N],
                            start=(c == 0),
                            stop=(c == NCHUNK - 1),
                        )

            def emit_pv_span(c, pt_sp, sp):
                if PV_OFF:
                    return
                for qs in range(span // MM_N):
                    q0 = sp * span + qs * MM_N
                    nc.tensor.matmul(
                        acc[:, q0 : q0 + MM_N],
                        lhsT=vp_sb[:, c, :],
                        rhs=pt_sp[:, qs * MM_N : (qs + 1) * MM_N],
                        start=(c == 0),
                        stop=(c == NCHUNK - 1),
                    )

            pending = []  # [(chunk, [pt tiles per span])] awaiting PV emission
            for c in range(NCHUNK):
                if PV_ONCE:
                    # Retire each pending chunk's PV exactly once, PV_DELAY
                    # chunks after its QK; near the head tail retire two per
                    # chunk (and chunk 15 retires itself in-place) so the
                    # next head's first QK isn't stuck behind a PV burst.
                    if c >= NCHUNK - 3:
                        n_ret = min(2, len(pending))
                    else:
                        n_ret = 1 if len(pending) >= PV_DELAY else 0
                else:
                    n_ret = 0
                # with QK_PACK, chunk c runs on PE rows 0-63 (tile T0) and
                # chunk c^1 on rows 64-127 (tile T8), concurrently
                r0 = 64 * (c % 2) if QK_PACK else 0
                pts = []
                ptw = None
                if MASK_WIDE:
                    ptw = pt_pool.tile(
                        [P, S], DT16, tag="pt", name=f"ptw{h}_{c}", bufs=PV_DELAY + 3
                    )
                extra_ret = (
                    PV_ILV
                    and PV_TAIL
                    and c in (NCHUNK - 3, NCHUNK - 2)
                    and len(pending) > 1
                )
                for sp in range(spans):
                    if PV_ILV and PV_FIRST and not PV_ONCE and pending:
                        emit_pv_span(pending[0][0], pending[0][1][sp], sp)
                    st = qk_psum.tile([P, span], score_dt, tag="st", name=f"st{h}_{c}_{sp}")
                    if warm and h == 0 and c == 0 and sp == 0:
                        for _w in range(WARM):
                            nc.tensor.matmul(
                                st[:, 0:qk_n],
                                lhsT=warm_ab[0][:],
                                rhs=warm_ab[1][:],
                                start=True,
                                stop=True,
                            )
                    for j in range(span // qk_n):
                        q0 = sp * span + j * qk_n
                        if QK_OFF:
                            continue
                        nc.tensor.matmul(
                            st[:, j * qk_n : (j + 1) * qk_n],
                            lhsT=kT_sb[r0 : r0 + 64, c * P : (c + 1) * P],
                            rhs=qT_sb[r0 : r0 + 64, q0 : q0 + qk_n],
                            start=True,
                            stop=True,
                        )
                    if MASK_WIDE:
                        pt = ptw[:, sp * span : (sp + 1) * span]
                    else:
                        pt = pt_pool.tile([P, span], DT16, tag="pt", name=f"pt{h}_{c}_{sp}")
                    nc.scalar.activation(pt[:], st[:], mybir.ActivationFunctionType.Exp)
                    if MASK_WIDE:
                        pts.append(pt)
                        if sp == spans - 1 and not MASK_OFF:
                            nc.vector.tensor_mul(ptw[:], ptw[:], maskT_sb[c][:])
                        continue
                    if not MASK_OFF:
                        if MASK_SEP:
                            ptm = pt_pool.tile(
                                [P, span], DT16, tag="ptm", name=f"ptm{h}_{c}_{sp}"
                            )
                            nc.vector.tensor_mul(
                                ptm[:], pt[:], maskT_sb[c][:, sp * span : (sp + 1) * span]
                            )
                            pt = ptm
                        else:
                            eng = nc.gpsimd if (MASK_GPS and sp % 2 == 1) else nc.vector
                            eng.tensor_mul(
                                pt[:], pt[:], maskT_sb[c][:, sp * span : (sp + 1) * span]
                            )
                    pts.append(pt)
                    if PV_ILV and PV_ONCE:
                        for item in pending[:n_ret]:
                            emit_pv_span(item[0], item[1][sp], sp)
                        if c == NCHUNK - 1:
                            emit_pv_span(c, pt, sp)
                    elif PV_ILV and pending:
                        if not PV_FIRST:
                            emit_pv_span(pending[0][0], pending[0][1][sp], sp)
                        if extra_ret:
                            emit_pv_span(pending[1][0], pending[1][1][sp], sp)
                        if PV_TAIL and c == NCHUNK - 1:
                            emit_pv_span(c, pt, sp)
                if OUT_SPLIT and c in (0, 1):
                    flush_deferred(c)
                if PV_ONCE:
                    del pending[:n_ret]
                    if c != NCHUNK - 1:
                        pending.append((c, pts))
                    continue
                if PV_ILV and PV_TAIL:
                    if extra_ret:
                        del pending[:2]  # both emitted this chunk
                        pending.append((c, pts))
                        continue
                    if c == NCHUNK - 1:
                        # pending[0] (c14) emitted via the normal path above,
                        # c15 self-emitted in-place: nothing left to flush
                        pending.clear()
                        continue
                pending.append((c, pts))
                if len(pending) > PV_DELAY:
                    done = pending.pop(0)
                    if not PV_ILV:
                        emit_pv(*done)
            for item in pending:
                if PV_ILV:
                    for sp in range(spans):
                        emit_pv_span(item[0], item[1][sp], sp)
                else:
                    emit_pv(*item)
            out_sb = out_pool.tile([D + 1, S], F32, tag="out", name=f"out_sb{h}")
            if PV_OFF:
                nc.gpsimd.memset(out_sb[:], 0.0)
                (nc.gpsimd if AUX_GPS else nc.sync).dma_start(o[h], out_sb[:])
            elif OUT_SPLIT:
                deferred.append((acc, out_sb, h))
                if h == HPC - 1:
                    flush_deferred(0)
                    flush_deferred(1)
            else:
                if OUT_ENG == "scalar":
                    nc.scalar.copy(out_sb[:], acc[:])
                else:
                    out_eng().tensor_copy(out_sb[:], acc[:])
                (nc.gpsimd if AUX_GPS else nc.sync).dma_start(o[h], out_sb[:])

        if loop_reps is None:
            body()
        else:
            if MASK_DMA_ONCE:
                hoisted[0] = load_mask()
            assert loop_reps % UNROLL == 0, (loop_reps, UNROLL)
            if STAGGER:
                with tc.For_i(0, loop_reps // UNROLL, 1, staggered_reset=True) as _i:
                    for _u in range(UNROLL):
                        body(_i)
            else:
                hints = ()
                if LOOP_HINTS:
                    hints = tuple(mybir.ALL_ENGINES)
                with tc.For_i(0, loop_reps // UNROLL, 1, hint_engines=hints) as _i:
                    for _u in range(UNROLL):
                        body(_i)

    nc.compile()
    return nc


def _get_nc():
    if "nc" not in _CACHE:
        _CACHE["nc"] = _build_nc()
    return _CACHE["nc"]


def _prep_inputs(q, k, v, mask):
    """Host-side shard + layout prep. Returns one input map per core."""
    np16 = np.float16 if PREC == "fp16" else ml_dtypes.bfloat16
    qk_np_dt = np.float32 if QK_DT in ("f32", "f32r") else np16
    q = np.asarray(q, dtype=np.float32)
    k = np.asarray(k, dtype=np.float32)
    v = np.asarray(v, dtype=np.float32)
    mask = np.asarray(mask)

    # [B,H,S,D] -> [B*H, ...]
    qf = q.reshape(B * H, S, D)
    kf = k.reshape(B * H, S, D)
    vf = v.reshape(B * H, S, D)

    # transposed layouts; fold the 1/sqrt(D) scale into q before rounding
    qTf = np.ascontiguousarray(np.transpose(qf / np.sqrt(np.float32(D)), (0, 2, 1))).astype(qk_np_dt)  # [BH, 64, S]
    kTf = np.ascontiguousarray(np.transpose(kf, (0, 2, 1))).astype(qk_np_dt)
    if QK_PACK:
        # duplicate rows so chunk pairs can use PE row-tiles T0/T8
        qTf = np.concatenate([qTf, qTf], axis=1)  # [BH, 128, S]
        kTf = np.concatenate([kTf, kTf], axis=1)
    ones = np.ones((B * H, S, 1), np.float32)
    vpf = np.concatenate([vf, ones], axis=2).astype(np16)  # [BH, S, 65]
    maskT = np.ascontiguousarray(mask[0, 0].T).astype(np16)  # [S, S]

    in_maps = []
    for ci in range(N_CORES):
        sl = slice(ci * HPC, (ci + 1) * HPC)
        if QK_MERGE:
            m = {
                "qk": np.ascontiguousarray(
                    np.concatenate([qTf[sl], kTf[sl]], axis=2)
                ),
                "vp": np.ascontiguousarray(vpf[sl]),
                "maskT": maskT,
            }
        else:
            m = {
                "qT": np.ascontiguousarray(qTf[sl]),
                "kT": np.ascontiguousarray(kTf[sl]),
                "vp": np.ascontiguousarray(vpf[sl]),
                "maskT": maskT,
            }
        in_maps.append(m)
    return in_maps


def kernel(q, k, v, mask):
    global LAST_RESULT
    from concourse import bass_utils

    nc = _get_nc()
    in_maps = _prep_inputs(q, k, v, mask)
    res = bass_utils.run_bass_kernel_spmd(
        nc, in_maps, core_ids=list(range(N_CORES))
    )
    LAST_RESULT = res

    out = np.empty((B * H, S, D), np.float32)
    for ci in range(N_CORES):
        oc = res.results[ci]["o"]  # [HPC, 65, S] f32
        num = oc[:, :D, :]  # (P@V)^T
        den = oc[:, D : D + 1, :]  # l
        out[ci * HPC : (ci + 1) * HPC] = np.transpose(num / den, (0, 2, 1))
    return out.reshape(B, H, S, D)

